# revision 7
# baseline (speedup 1.0000x reference)
"""Trainium2 Bass kernel for nn_MEPG_Loss (MEPG policy-gradient loss).

Math (forward only; stop_gradient is identity):
    h   = tanh(states[s,:,t] @ W1 + b1)                  [S,T,H]
    mu  = h @ W2 + b2                                    [S,T,A]
    ll[s,t] = -0.5*(||a[s,:,t]-mu||^2/SD + A*log(2*pi*SD))
    base = rewards.T - ALPHA*ll.T ; cum = base with row T-2 += row T-1
    A_hat = cum - log(0.5)
    out = einsum('ts,us->', A_hat, ll.T)/S
        = sum_s (sum_t A_hat[t,s]) * (sum_t ll[t,s]) / S

Only per-simulation reductions are needed:
    q_sum[s]  = sum_{t,d} (mu - a)^2,   q_last[s] = sum_d (mu - a)^2 at t=T-1
    R[s] = sum_t rewards,               r_last[s] = rewards[s,T-1]
(R/r_last come straight from host numpy; rewards are never sent to the device.)

Device pipeline, per core (256 sims as 64 quads of 4 sims):
    - states prepacked on host to [64, NQ*T] bf16; per 4-quad block one
      contiguous DMA per sim-slot j lands at SBUF partitions {32j..32j+16}
    - mm1: 4 row-tiled K=16 matmuls (concurrent via tile_position) fill a
      4-bank PSUM tile hp [128, 4*T] with h_pre for the whole quad
    - ScalarE: ONE tanh activation over all 2048 columns (bias=b1) -> h bf16
    - mm2: 4 col-tiled matmuls (lhsT=W2, start) write mu into hp[:, 0:T],
      reusing the first bank of the already-consumed h_pre tile
    - diff: 4 diag-tiled identity matmuls accumulate (b2 - a) onto mu
    - DVE: bn_stats + bn_aggr give per-partition mean/var of diff over T
      (=> sum of squares), plus a 1-col copy of diff at t=T-1
Final combine (tiny) is done on host in float64.

ScalarE is the bottleneck engine (tanh, 1 elem/lane/cycle): ~118 us floor.
Everything else is sized to stay below that and overlap fully.
"""

import os
import sys

import numpy as np

if not any(os.path.isdir(os.path.join(p, "concourse")) for p in sys.path if p):
    sys.path.insert(0, "/opt/trn_rl_repo")

import ml_dtypes

import concourse.bacc as bacc
import concourse.tile as tile
from concourse import mybir
from concourse.bass_utils import run_bass_kernel_spmd

# Problem constants (hardcoded per contract)
S, D, A, T, HID = 2048, 16, 4, 512, 128
N_CORES = 8
SS = S // N_CORES          # 256 sims per core
NQ = SS // 4               # 64 quads per core
QB = 4                     # quads per DMA block
NB = NQ // QB              # 16 blocks
SD_VAR = 0.04
ALPHA = 0.1
MAX_POSITION = 1.0

F32 = mybir.dt.float32
BF16 = mybir.dt.bfloat16
NP_BF16 = ml_dtypes.bfloat16


def _build_program():
    nc = bacc.Bacc("TRN2", target_bir_lowering=False, debug=False)

    stp_d = nc.dram_tensor("st_pre", [64, NQ * T], BF16, kind="ExternalInput").ap()
    atp_d = nc.dram_tensor("at_pre", [16, NQ * T], BF16, kind="ExternalInput").ap()
    w1f_d = nc.dram_tensor("w1full", [128, HID], BF16, kind="ExternalInput").ap()
    w2_d = nc.dram_tensor("w2", [HID, A], BF16, kind="ExternalInput").ap()
    id4_d = nc.dram_tensor("id4", [128, A], BF16, kind="ExternalInput").ap()
    b1_d = nc.dram_tensor("b1col", [HID, 1], F32, kind="ExternalInput").ap()

    mv_d = nc.dram_tensor("mv", [128, 2 * NQ], F32, kind="ExternalOutput").ap()
    ql_d = nc.dram_tensor("ql", [128, NQ], F32, kind="ExternalOutput").ap()

    with tile.TileContext(nc) as tc:
        with (
            tc.tile_pool(name="consts", bufs=1) as consts,
            tc.tile_pool(name="stp", bufs=3) as stp,
            tc.tile_pool(name="atp", bufs=3) as atp,
            tc.tile_pool(name="hsb", bufs=2) as hsb,
            tc.tile_pool(name="bstp", bufs=2) as bstp,
            tc.tile_pool(name="outs", bufs=1) as outp,
            tc.tile_pool(name="hpp", bufs=1, space="PSUM") as hpp,
        ):
            # One persistent PSUM tile covering all 8 banks, hand-carved:
            #   3 h_pre units of 2 banks (1024 f32) -> ACT is their ONLY
            #   reader, so mm1 of quad g never waits on the mu chain
            #   2 mu buffers of 1 bank each
            PS = hpp.tile([128, 4096], F32, tag="PS")
            UN = [PS[:, 1024 * u:1024 * (u + 1)] for u in range(3)]
            MU = [PS[:, 3072:3584], PS[:, 3584:4096]]
            # dummy activation: forces the tanh table load at t~0, off the
            # critical path (no data dependence)
            dums = consts.tile([128, 1], F32, tag="dums")
            dumo = consts.tile([128, 1], F32, tag="dumo")
            nc.vector.memset(dums[:], 0.0)
            nc.scalar.activation(
                out=dumo[:], in_=dums[:],
                func=mybir.ActivationFunctionType.Tanh, scale=1.0,
            )

            # constants
            w1t = consts.tile([128, HID], BF16, tag="w1t")
            w2t = consts.tile([HID, A], BF16, tag="w2t")
            id4t = consts.tile([128, A], BF16, tag="id4t")
            b1t = consts.tile([HID, 1], F32, tag="b1t")
            nc.sync.dma_start(out=w1t[:], in_=w1f_d)
            nc.sync.dma_start(out=w2t[:], in_=w2_d)
            nc.sync.dma_start(out=id4t[:], in_=id4_d)
            nc.sync.dma_start(out=b1t[:], in_=b1_d)

            mv_sb = outp.tile([128, 2 * NQ], F32, tag="mv")
            ql_sb = outp.tile([128, NQ], F32, tag="ql")

            def _tail_quad(g, h, at):
                q = g % QB
                mu = MU[g % 2]
                for j in range(4):
                    nc.tensor.matmul(
                        out=mu[32 * j:32 * j + A, :],
                        lhsT=w2t[:],
                        rhs=h[:, T * j:T * (j + 1)],
                        start=True, stop=False,
                        tile_position=(0, 32 * j),
                        skip_group_check=True,
                    )
                for j in range(4):
                    nc.tensor.matmul(
                        out=mu[32 * j:32 * j + A, :],
                        lhsT=id4t[32 * j:32 * j + A, :],
                        rhs=at[32 * j:32 * j + A, T * q:T * (q + 1)],
                        start=False, stop=True,
                        tile_position=(32 * j, 32 * j),
                        skip_group_check=True,
                    )
                sts = bstp.tile([128, 6], F32, tag="bst", name=f"bst_{g}")
                nc.vector.tensor_copy(ql_sb[:, g:g + 1], mu[:, T - 1:T])
                nc.vector.bn_stats(out=sts[:], in_=mu[:])
                nc.vector.bn_aggr(out=mv_sb[:, 2 * g:2 * g + 2], in_=sts[:])

            pipe = None
            for b in range(NB):
                c0 = QB * T * b
                st = stp.tile([128, QB * T], BF16, tag="st", name=f"st_{b}")
                at = atp.tile([128, QB * T], BF16, tag="at", name=f"at_{b}")
                # all DMAs via HWDGE (sync); gpsimd SWDGE costs ~600ns of Q7
                # descriptor-gen per dma_start and paces the whole stream
                for j in range(4):
                    nc.sync.dma_start(
                        out=st[32 * j:32 * j + D, :],
                        in_=stp_d[D * j:D * (j + 1), c0:c0 + QB * T],
                    )
                for j in range(4):
                    nc.sync.dma_start(
                        out=at[32 * j:32 * j + A, :],
                        in_=atp_d[A * j:A * (j + 1), c0:c0 + QB * T],
                    )
                for q in range(QB):
                    g = QB * b + q
                    ua, ub = (2 * g) % 3, (2 * g + 1) % 3
                    # mm1(g) first: its psum unit was freed by an ACT 1.5
                    # quads ago, so it starts immediately during ACT(g-1)
                    for j in range(4):
                        dst = UN[ua] if j < 2 else UN[ub]
                        nc.tensor.matmul(
                            out=dst[:, T * (j % 2):T * (j % 2 + 1)],
                            lhsT=w1t[32 * j:32 * j + D, :],
                            rhs=st[32 * j:32 * j + D, T * q:T * (q + 1)],
                            start=True, stop=True,
                            tile_position=(32 * j, 0),
                        )
                    h = hsb.tile([128, 4 * T], BF16, tag="h", name=f"h_{g}")
                    if ub == ua + 1:
                        nc.scalar.activation(
                            out=h[:], in_=PS[:, 1024 * ua:1024 * ua + 2048],
                            func=mybir.ActivationFunctionType.Tanh,
                            bias=b1t[:], scale=1.0,
                        )
                    else:
                        nc.scalar.activation(
                            out=h[:, 0:2 * T], in_=UN[ua],
                            func=mybir.ActivationFunctionType.Tanh,
                            bias=b1t[:], scale=1.0,
                        )
                        nc.scalar.activation(
                            out=h[:, 2 * T:4 * T], in_=UN[ub],
                            func=mybir.ActivationFunctionType.Tanh,
                            bias=b1t[:], scale=1.0,
                        )
                    if pipe is not None:
                        _tail_quad(*pipe)
                    pipe = (g, h, at)

            if pipe is not None:
                _tail_quad(*pipe)

            nc.sync.dma_start(out=mv_d, in_=mv_sb[:])
            nc.sync.dma_start(out=ql_d, in_=ql_sb[:])

    nc.finalize()
    return nc


_NC_CACHE = {}


def _get_program():
    if "nc" not in _NC_CACHE:
        _NC_CACHE["nc"] = _build_program()
    return _NC_CACHE["nc"]


def _make_consts(W1, b1, W2):
    w1full = np.zeros((128, HID), dtype=NP_BF16)
    id4 = np.zeros((128, A), dtype=NP_BF16)
    for j in range(4):
        w1full[32 * j:32 * j + D, :] = W1.astype(NP_BF16)
        for d in range(A):
            id4[32 * j + d, d] = 1.0
    return {
        "w1full": w1full,
        "w2": np.ascontiguousarray(W2.astype(NP_BF16)),
        "id4": id4,
        "b1col": np.ascontiguousarray(b1.astype(np.float32).reshape(HID, 1)),
    }


def kernel(states, actions, rewards, W1, b1, W2, b2, _run_kwargs=None):
    states = np.asarray(states, dtype=np.float32)
    actions = np.asarray(actions, dtype=np.float32)
    rewards = np.asarray(rewards, dtype=np.float32)
    W1 = np.asarray(W1, dtype=np.float32)
    b1 = np.asarray(b1, dtype=np.float32)
    W2 = np.asarray(W2, dtype=np.float32)
    b2 = np.asarray(b2, dtype=np.float32)

    consts = _make_consts(W1, b1, W2)

    # prepack per-core device layouts:
    #   st_pre[16j+dd, g*T+t] = states[core*SS + 4g+j, dd, t]   (bf16)
    #   at_pre[4j+d,  g*T+t] = b2[d] - actions[core*SS + 4g+j, d, t]  (bf16)
    st_all = states.reshape(N_CORES, SS // 4, 4, D, T)
    st_all = np.ascontiguousarray(st_all.transpose(0, 2, 3, 1, 4)).astype(NP_BF16)
    st_all = st_all.reshape(N_CORES, 64, NQ * T)
    aadj = b2[None, :, None] - actions
    at_all = aadj.reshape(N_CORES, SS // 4, 4, A, T)
    at_all = np.ascontiguousarray(at_all.transpose(0, 2, 3, 1, 4)).astype(NP_BF16)
    at_all = at_all.reshape(N_CORES, 16, NQ * T)

    in_maps = []
    for c in range(N_CORES):
        m = {"st_pre": st_all[c], "at_pre": at_all[c]}
        m.update(consts)
        in_maps.append(m)

    nc = _get_program()
    res = run_bass_kernel_spmd(nc, in_maps, core_ids=list(range(N_CORES)),
                               **(_run_kwargs or {}))
    results = res.results

    # host combine in float64
    C0 = -0.5 * A * np.log(2.0 * np.pi * SD_VAR)
    mx_pos = np.log(1.0 / (2.0 * MAX_POSITION))
    rew = rewards.astype(np.float64)
    R_all = rew.sum(axis=1)            # [S]
    rlast_all = rew[:, -1]             # [S]
    total = 0.0
    for c in range(N_CORES):
        mv = results[c]["mv"].astype(np.float64)      # [128, 2*NQ]
        qlv = results[c]["ql"].astype(np.float64)     # [128, NQ]
        mean = mv[:, 0::2]                            # [128, NQ]
        var = mv[:, 1::2]
        sumsq = T * (var + mean * mean)               # Sum_t diff^2 per (p, g)
        # partition p = 32j + d (d < A), sim s_local = 4g + j
        sel = sumsq.reshape(4, 32, NQ)[:, :A, :]      # [j, d, g]
        q_sum = sel.sum(axis=1).T.reshape(SS)         # s_local = 4g + j
        qsel = qlv.reshape(4, 32, NQ)[:, :A, :]
        q_last = (qsel ** 2).sum(axis=1).T.reshape(SS)
        sl = slice(SS * c, SS * (c + 1))
        L = -0.5 * q_sum / SD_VAR + T * C0
        ll_last = -0.5 * q_last / SD_VAR + C0
        A_sum = (R_all[sl] + rlast_all[sl]
                 - ALPHA * (L + ll_last) - T * mx_pos)
        total += np.sum(A_sum * L)
    out = np.float32(total / S)
    if _run_kwargs:
        _NC_CACHE["last_result"] = res
    return out


if __name__ == "__main__":
    rng = np.random.default_rng(0)
    inputs = {
        "states": rng.standard_normal((S, D, T), dtype=np.float32),
        "actions": rng.standard_normal((S, A, T), dtype=np.float32),
        "rewards": rng.standard_normal((S, T), dtype=np.float32),
        "W1": (rng.standard_normal((D, HID)) / np.sqrt(D)).astype(np.float32),
        "b1": np.zeros(HID, np.float32),
        "W2": (rng.standard_normal((HID, A)) / np.sqrt(HID)).astype(np.float32),
        "b2": np.zeros(A, np.float32),
    }
    print("result:", kernel(**inputs))


# revision 10
# speedup vs baseline: 1.0562x; 1.0562x over previous
"""Trainium2 Bass kernel for nn_MEPG_Loss (MEPG policy-gradient loss).

Math (forward only; stop_gradient is identity):
    h   = tanh(states[s,:,t] @ W1 + b1)                  [S,T,H]
    mu  = h @ W2 + b2                                    [S,T,A]
    ll[s,t] = -0.5*(||a[s,:,t]-mu||^2/SD + A*log(2*pi*SD))
    base = rewards.T - ALPHA*ll.T ; cum = base with row T-2 += row T-1
    A_hat = cum - log(0.5)
    out = einsum('ts,us->', A_hat, ll.T)/S
        = sum_s (sum_t A_hat[t,s]) * (sum_t ll[t,s]) / S

Only per-simulation reductions are needed:
    q_sum[s]  = sum_{t,d} (mu - a)^2,   q_last[s] = sum_d (mu - a)^2 at t=T-1
    R[s] = sum_t rewards,               r_last[s] = rewards[s,T-1]
(R/r_last come straight from host numpy; rewards are never sent to the device.)

Device pipeline, per core (256 sims as 64 quads of 4 sims):
    - states prepacked on host to [64, NQ*T] bf16; per 4-quad block one
      contiguous DMA per sim-slot j lands at SBUF partitions {32j..32j+16}
    - mm1: 4 row-tiled K=16 matmuls (concurrent via tile_position) fill a
      4-bank PSUM tile hp [128, 4*T] with h_pre for the whole quad
    - ScalarE: ONE tanh activation over all 2048 columns (bias=b1) -> h bf16
    - mm2: 4 col-tiled matmuls (lhsT=W2, start) write mu into hp[:, 0:T],
      reusing the first bank of the already-consumed h_pre tile
    - diff: 4 diag-tiled identity matmuls accumulate (b2 - a) onto mu
    - DVE: bn_stats + bn_aggr give per-partition mean/var of diff over T
      (=> sum of squares), plus a 1-col copy of diff at t=T-1
Final combine (tiny) is done on host in float64.

ScalarE is the bottleneck engine (tanh, 1 elem/lane/cycle): ~118 us floor.
Everything else is sized to stay below that and overlap fully.
"""

import os
import sys

import numpy as np

if not any(os.path.isdir(os.path.join(p, "concourse")) for p in sys.path if p):
    sys.path.insert(0, "/opt/trn_rl_repo")

import ml_dtypes

import concourse.bacc as bacc
import concourse.tile as tile
from concourse import mybir
from concourse.bass_utils import run_bass_kernel_spmd

# Problem constants (hardcoded per contract)
S, D, A, T, HID = 2048, 16, 4, 512, 128
N_CORES = 8
SS = S // N_CORES          # 256 sims per core
NQ = SS // 4               # 64 quads per core
QB = 4                     # quads per DMA block
NB = NQ // QB              # 16 blocks
SD_VAR = 0.04
ALPHA = 0.1
MAX_POSITION = 1.0

F32 = mybir.dt.float32
BF16 = mybir.dt.bfloat16
NP_BF16 = ml_dtypes.bfloat16


def _build_program():
    nc = bacc.Bacc("TRN2", target_bir_lowering=False, debug=False)

    stp_d = nc.dram_tensor("st_pre", [64, NQ * T], BF16, kind="ExternalInput").ap()
    atp_d = nc.dram_tensor("at_pre", [16, NQ * T], BF16, kind="ExternalInput").ap()
    w1f_d = nc.dram_tensor("w1full", [128, HID], BF16, kind="ExternalInput").ap()
    w2_d = nc.dram_tensor("w2", [HID, A], BF16, kind="ExternalInput").ap()
    id4_d = nc.dram_tensor("id4", [128, A], BF16, kind="ExternalInput").ap()
    b1_d = nc.dram_tensor("b1col", [HID, 1], F32, kind="ExternalInput").ap()

    mv_d = nc.dram_tensor("mv", [128, 2 * NQ], F32, kind="ExternalOutput").ap()
    ql_d = nc.dram_tensor("ql", [128, NQ], F32, kind="ExternalOutput").ap()

    with tile.TileContext(nc) as tc:
        with (
            tc.tile_pool(name="consts", bufs=1) as consts,
            tc.tile_pool(name="stp", bufs=4) as stp,
            tc.tile_pool(name="atp", bufs=4) as atp,
            tc.tile_pool(name="hsb", bufs=2) as hsb,
            tc.tile_pool(name="bstp", bufs=2) as bstp,
            tc.tile_pool(name="outs", bufs=1) as outp,
            tc.tile_pool(name="hpp", bufs=1, space="PSUM") as hpp,
        ):
            # One persistent PSUM tile covering all 8 banks, hand-carved:
            #   3 h_pre units of 2 banks (1024 f32) -> ACT is their ONLY
            #   reader, so mm1 of quad g never waits on the mu chain
            #   2 mu buffers of 1 bank each
            PS = hpp.tile([128, 4096], F32, tag="PS")
            UN = [PS[:, 1024 * u:1024 * (u + 1)] for u in range(3)]
            MU = [PS[:, 3072:3584], PS[:, 3584:4096]]
            # dummy activation: forces the tanh table load at t~0, off the
            # critical path (no data dependence)
            dums = consts.tile([128, 1], F32, tag="dums")
            dumo = consts.tile([128, 1], F32, tag="dumo")
            nc.vector.memset(dums[:], 0.0)
            nc.scalar.activation(
                out=dumo[:], in_=dums[:],
                func=mybir.ActivationFunctionType.Tanh, scale=1.0,
            )

            # constants
            w1t = consts.tile([128, HID], BF16, tag="w1t")
            w2t = consts.tile([HID, A], BF16, tag="w2t")
            id4t = consts.tile([128, A], BF16, tag="id4t")
            b1t = consts.tile([HID, 1], F32, tag="b1t")
            nc.sync.dma_start(out=w1t[:], in_=w1f_d)
            nc.sync.dma_start(out=w2t[:], in_=w2_d)
            nc.sync.dma_start(out=id4t[:], in_=id4_d)
            nc.sync.dma_start(out=b1t[:], in_=b1_d)

            mv_sb = outp.tile([128, 2 * NQ], F32, tag="mv")
            ql_sb = outp.tile([128, NQ], F32, tag="ql")

            def _tail_quad(g, h, at):
                q = g % QB
                mu = MU[g % 2]
                for j in range(4):
                    nc.tensor.matmul(
                        out=mu[32 * j:32 * j + A, :],
                        lhsT=w2t[:],
                        rhs=h[:, T * j:T * (j + 1)],
                        start=True, stop=False,
                        tile_position=(0, 32 * j),
                        skip_group_check=True,
                    )
                for j in range(4):
                    nc.tensor.matmul(
                        out=mu[32 * j:32 * j + A, :],
                        lhsT=id4t[32 * j:32 * j + A, :],
                        rhs=at[32 * j:32 * j + A, T * q:T * (q + 1)],
                        start=False, stop=True,
                        tile_position=(32 * j, 32 * j),
                        skip_group_check=True,
                    )
                sts = bstp.tile([128, 6], F32, tag="bst", name=f"bst_{g}")
                nc.vector.tensor_copy(ql_sb[:, g:g + 1], mu[:, T - 1:T])
                nc.vector.bn_stats(out=sts[:], in_=mu[:])
                nc.vector.bn_aggr(out=mv_sb[:, 2 * g:2 * g + 2], in_=sts[:])

            # block DMA management: explicit 2-block prefetch, all HWDGE
            blocks = {}

            def _ensure_block(b):
                if b in blocks or b >= NB:
                    return
                c0 = QB * T * b
                st = stp.tile([128, QB * T], BF16, tag="st", name=f"st_{b}")
                at = atp.tile([128, QB * T], BF16, tag="at", name=f"at_{b}")
                for j in range(4):
                    nc.sync.dma_start(
                        out=st[32 * j:32 * j + D, :],
                        in_=stp_d[D * j:D * (j + 1), c0:c0 + QB * T],
                    )
                for j in range(4):
                    nc.sync.dma_start(
                        out=at[32 * j:32 * j + A, :],
                        in_=atp_d[A * j:A * (j + 1), c0:c0 + QB * T],
                    )
                blocks[b] = (st, at)

            def _mm1(g):
                st, _ = blocks[g // QB]
                q = g % QB
                ua, ub = (2 * g) % 3, (2 * g + 1) % 3
                for j in range(4):
                    dst = UN[ua] if j < 2 else UN[ub]
                    nc.tensor.matmul(
                        out=dst[:, T * (j % 2):T * (j % 2 + 1)],
                        lhsT=w1t[32 * j:32 * j + D, :],
                        rhs=st[32 * j:32 * j + D, T * q:T * (q + 1)],
                        start=True, stop=True,
                        tile_position=(32 * j, 0),
                    )

            def _act(g):
                ua, ub = (2 * g) % 3, (2 * g + 1) % 3
                h = hsb.tile([128, 4 * T], BF16, tag="h", name=f"h_{g}")
                if ub == ua + 1:
                    nc.scalar.activation(
                        out=h[:], in_=PS[:, 1024 * ua:1024 * ua + 2048],
                        func=mybir.ActivationFunctionType.Tanh,
                        bias=b1t[:], scale=1.0,
                    )
                else:
                    nc.scalar.activation(
                        out=h[:, 0:2 * T], in_=UN[ua],
                        func=mybir.ActivationFunctionType.Tanh,
                        bias=b1t[:], scale=1.0,
                    )
                    nc.scalar.activation(
                        out=h[:, 2 * T:4 * T], in_=UN[ub],
                        func=mybir.ActivationFunctionType.Tanh,
                        bias=b1t[:], scale=1.0,
                    )
                return h

            _ensure_block(0)
            _ensure_block(1)
            _ensure_block(2)
            # software pipeline, 2 quads deep on mm1: at quad g we emit
            # mm1(g+1) FIRST (its psum units were freed by ACT(g-1), so it
            # fills during ACT(g)), then ACT(g), then the mu/bn tail of g-1.
            _mm1(0)
            hprev = None
            for g in range(NQ):
                if g + 1 < NQ:
                    if (g + 1) % QB == 0:
                        _ensure_block((g + 1) // QB + 2)
                    _mm1(g + 1)
                h = _act(g)
                if hprev is not None:
                    _tail_quad(g - 1, hprev, blocks[(g - 1) // QB][1])
                hprev = h
                # stream outputs every 16 quads to keep the tail short
                if g % 16 == 8 and g > 16:
                    k = g // 16 - 1
                    nc.sync.dma_start(out=mv_d[:, 32 * k:32 * (k + 1)],
                                      in_=mv_sb[:, 32 * k:32 * (k + 1)])
                    nc.sync.dma_start(out=ql_d[:, 16 * k:16 * (k + 1)],
                                      in_=ql_sb[:, 16 * k:16 * (k + 1)])
            _tail_quad(NQ - 1, hprev, blocks[(NQ - 1) // QB][1])

            k = 3
            nc.sync.dma_start(out=mv_d[:, 32 * k:32 * (k + 1)],
                              in_=mv_sb[:, 32 * k:32 * (k + 1)])
            nc.sync.dma_start(out=ql_d[:, 16 * k:16 * (k + 1)],
                              in_=ql_sb[:, 16 * k:16 * (k + 1)])

    nc.finalize()
    return nc


_NC_CACHE = {}


def _get_program():
    if "nc" not in _NC_CACHE:
        _NC_CACHE["nc"] = _build_program()
    return _NC_CACHE["nc"]


def _make_consts(W1, b1, W2):
    w1full = np.zeros((128, HID), dtype=NP_BF16)
    id4 = np.zeros((128, A), dtype=NP_BF16)
    for j in range(4):
        w1full[32 * j:32 * j + D, :] = W1.astype(NP_BF16)
        for d in range(A):
            id4[32 * j + d, d] = 1.0
    return {
        "w1full": w1full,
        "w2": np.ascontiguousarray(W2.astype(NP_BF16)),
        "id4": id4,
        "b1col": np.ascontiguousarray(b1.astype(np.float32).reshape(HID, 1)),
    }


def kernel(states, actions, rewards, W1, b1, W2, b2, _run_kwargs=None):
    states = np.asarray(states, dtype=np.float32)
    actions = np.asarray(actions, dtype=np.float32)
    rewards = np.asarray(rewards, dtype=np.float32)
    W1 = np.asarray(W1, dtype=np.float32)
    b1 = np.asarray(b1, dtype=np.float32)
    W2 = np.asarray(W2, dtype=np.float32)
    b2 = np.asarray(b2, dtype=np.float32)

    consts = _make_consts(W1, b1, W2)

    # prepack per-core device layouts:
    #   st_pre[16j+dd, g*T+t] = states[core*SS + 4g+j, dd, t]   (bf16)
    #   at_pre[4j+d,  g*T+t] = b2[d] - actions[core*SS + 4g+j, d, t]  (bf16)
    st_all = states.reshape(N_CORES, SS // 4, 4, D, T)
    st_all = np.ascontiguousarray(st_all.transpose(0, 2, 3, 1, 4)).astype(NP_BF16)
    st_all = st_all.reshape(N_CORES, 64, NQ * T)
    aadj = b2[None, :, None] - actions
    at_all = aadj.reshape(N_CORES, SS // 4, 4, A, T)
    at_all = np.ascontiguousarray(at_all.transpose(0, 2, 3, 1, 4)).astype(NP_BF16)
    at_all = at_all.reshape(N_CORES, 16, NQ * T)

    in_maps = []
    for c in range(N_CORES):
        m = {"st_pre": st_all[c], "at_pre": at_all[c]}
        m.update(consts)
        in_maps.append(m)

    nc = _get_program()
    res = run_bass_kernel_spmd(nc, in_maps, core_ids=list(range(N_CORES)),
                               **(_run_kwargs or {}))
    results = res.results

    # host combine in float64
    C0 = -0.5 * A * np.log(2.0 * np.pi * SD_VAR)
    mx_pos = np.log(1.0 / (2.0 * MAX_POSITION))
    rew = rewards.astype(np.float64)
    R_all = rew.sum(axis=1)            # [S]
    rlast_all = rew[:, -1]             # [S]
    total = 0.0
    for c in range(N_CORES):
        mv = results[c]["mv"].astype(np.float64)      # [128, 2*NQ]
        qlv = results[c]["ql"].astype(np.float64)     # [128, NQ]
        mean = mv[:, 0::2]                            # [128, NQ]
        var = mv[:, 1::2]
        sumsq = T * (var + mean * mean)               # Sum_t diff^2 per (p, g)
        # partition p = 32j + d (d < A), sim s_local = 4g + j
        sel = sumsq.reshape(4, 32, NQ)[:, :A, :]      # [j, d, g]
        q_sum = sel.sum(axis=1).T.reshape(SS)         # s_local = 4g + j
        qsel = qlv.reshape(4, 32, NQ)[:, :A, :]
        q_last = (qsel ** 2).sum(axis=1).T.reshape(SS)
        sl = slice(SS * c, SS * (c + 1))
        L = -0.5 * q_sum / SD_VAR + T * C0
        ll_last = -0.5 * q_last / SD_VAR + C0
        A_sum = (R_all[sl] + rlast_all[sl]
                 - ALPHA * (L + ll_last) - T * mx_pos)
        total += np.sum(A_sum * L)
    out = np.float32(total / S)
    if _run_kwargs:
        _NC_CACHE["last_result"] = res
    return out


if __name__ == "__main__":
    rng = np.random.default_rng(0)
    inputs = {
        "states": rng.standard_normal((S, D, T), dtype=np.float32),
        "actions": rng.standard_normal((S, A, T), dtype=np.float32),
        "rewards": rng.standard_normal((S, T), dtype=np.float32),
        "W1": (rng.standard_normal((D, HID)) / np.sqrt(D)).astype(np.float32),
        "b1": np.zeros(HID, np.float32),
        "W2": (rng.standard_normal((HID, A)) / np.sqrt(HID)).astype(np.float32),
        "b2": np.zeros(A, np.float32),
    }
    print("result:", kernel(**inputs))


# revision 11
# speedup vs baseline: 1.1571x; 1.0956x over previous
"""Trainium2 Bass kernel for nn_MEPG_Loss (MEPG policy-gradient loss).

Math (forward only; stop_gradient is identity):
    h   = tanh(states[s,:,t] @ W1 + b1)                  [S,T,H]
    mu  = h @ W2 + b2                                    [S,T,A]
    ll[s,t] = -0.5*(||a[s,:,t]-mu||^2/SD + A*log(2*pi*SD))
    out = sum_s (sum_t A_hat[t,s]) * (sum_t ll[t,s]) / S

Per-simulation reductions with v = W2^T h (device) and c = b2 - a (host):
    q_sum[s] = sum_t ||v+c||^2 = sum_t ||v||^2 + 2 sum_t <v,c> + sum_t ||c||^2
      - sum_t v, sum_t v^2 per partition: ONE bn_stats on the mu psum bank
      - cross = sum_t v*c per partition: ONE scalar_tensor_tensor accum
      - sum_t ||c||^2, rewards sums: host numpy (inputs are host-resident)
    q_last[s]: copy v[:, T-1], combine with host c[:, T-1]

Device pipeline, per core (256 sims as 64 quads of 4 sims):
    - states prepacked on host to [64, NQ*T] bf16; contiguous block DMAs
      (HWDGE via sync queue; gpsimd SWDGE costs ~600ns/DMA of Q7 time)
    - mm1: 4 row-tiled K=16 matmuls -> 2-bank psum units (3-unit rotation;
      the ACT is each unit's ONLY reader so mm1 runs 1.5 quads ahead)
    - ScalarE: one merged tanh over 2048 cols when the quad's two units are
      adjacent (2/3 of quads), else two 1024-col tanhs.  ScalarE is the
      bottleneck engine: ~1 elem/lane/cycle @ 1.2 GHz, ~127 us total.
    - mm2: 4 col-tiled matmuls (lhsT=W2) -> mu psum bank (x2 rotation)
    - DVE: stt cross-term + qlast copy + bn_stats + bn_aggr per quad
Final combine (tiny) in float64 on host.
"""

import os
import sys

import numpy as np

if not any(os.path.isdir(os.path.join(p, "concourse")) for p in sys.path if p):
    sys.path.insert(0, "/opt/trn_rl_repo")

import ml_dtypes

import concourse.bacc as bacc
import concourse.tile as tile
from concourse import mybir
from concourse.bass_utils import run_bass_kernel_spmd

# Problem constants (hardcoded per contract)
S, D, A, T, HID = 2048, 16, 4, 512, 128
N_CORES = 8
SS = S // N_CORES          # 256 sims per core
NQ = SS // 4               # 64 quads per core
SD_VAR = 0.04
ALPHA = 0.1
MAX_POSITION = 1.0

# DMA blocks of quads: small first blocks so the first mm1/tanh start early
BLOCK_SIZES = [1, 1, 2] + [4] * 15
BLOCK_Q0 = np.concatenate([[0], np.cumsum(BLOCK_SIZES)[:-1]]).tolist()
NBL = len(BLOCK_SIZES)
QUAD_BLOCK = []
for bi, (q0, nq) in enumerate(zip(BLOCK_Q0, BLOCK_SIZES)):
    QUAD_BLOCK += [bi] * nq

F32 = mybir.dt.float32
BF16 = mybir.dt.bfloat16
NP_BF16 = ml_dtypes.bfloat16


def _build_program():
    nc = bacc.Bacc("TRN2", target_bir_lowering=False, debug=False)

    stp_d = nc.dram_tensor("st_pre", [64, NQ * T], BF16, kind="ExternalInput").ap()
    atp_d = nc.dram_tensor("at_pre", [16, NQ * T], BF16, kind="ExternalInput").ap()
    w1f_d = nc.dram_tensor("w1full", [128, HID], BF16, kind="ExternalInput").ap()
    w2_d = nc.dram_tensor("w2", [HID, A], BF16, kind="ExternalInput").ap()
    b1_d = nc.dram_tensor("b1col", [HID, 1], F32, kind="ExternalInput").ap()

    mv_d = nc.dram_tensor("mv", [128, 2 * NQ], F32, kind="ExternalOutput").ap()
    ql_d = nc.dram_tensor("ql", [128, NQ], F32, kind="ExternalOutput").ap()
    cr_d = nc.dram_tensor("cr", [128, NQ], F32, kind="ExternalOutput").ap()

    with tile.TileContext(nc) as tc:
        with (
            tc.tile_pool(name="consts", bufs=1) as consts,
            tc.tile_pool(name="stp", bufs=4) as stp,
            tc.tile_pool(name="atp", bufs=4) as atp,
            tc.tile_pool(name="hsb", bufs=2) as hsb,
            tc.tile_pool(name="sdp", bufs=2) as sdp,
            tc.tile_pool(name="bstp", bufs=2) as bstp,
            tc.tile_pool(name="outs", bufs=1) as outp,
            tc.tile_pool(name="hpp", bufs=1, space="PSUM") as hpp,
        ):
            # One persistent PSUM tile covering all 8 banks, hand-carved:
            # 3 h_pre units of 2 banks + 2 mu banks (1 bank spare)
            PS = hpp.tile([128, 4096], F32, tag="PS")
            UN = [PS[:, 1024 * u:1024 * (u + 1)] for u in range(3)]
            MU = [PS[:, 3072:3584], PS[:, 3584:4096]]

            # dummy activation: forces the tanh table load at t~0
            dums = consts.tile([128, 1], F32, tag="dums")
            dumo = consts.tile([128, 1], F32, tag="dumo")
            nc.vector.memset(dums[:], 0.0)
            nc.scalar.activation(
                out=dumo[:], in_=dums[:],
                func=mybir.ActivationFunctionType.Tanh, scale=1.0,
            )

            # constants
            w1t = consts.tile([128, HID], BF16, tag="w1t")
            w2t = consts.tile([HID, A], BF16, tag="w2t")
            b1t = consts.tile([HID, 1], F32, tag="b1t")
            nc.sync.dma_start(out=w1t[:], in_=w1f_d)
            nc.sync.dma_start(out=w2t[:], in_=w2_d)
            nc.sync.dma_start(out=b1t[:], in_=b1_d)

            mv_sb = outp.tile([128, 2 * NQ], F32, tag="mv")
            ql_sb = outp.tile([128, NQ], F32, tag="ql")
            cr_sb = outp.tile([128, NQ], F32, tag="cr")

            blocks = {}

            def _ensure_block(bi):
                if bi in blocks or bi >= NBL:
                    return
                q0, nq = BLOCK_Q0[bi], BLOCK_SIZES[bi]
                c0 = T * q0
                st = stp.tile([128, nq * T], BF16, tag=f"st{nq}",
                              name=f"st_{bi}")
                at = atp.tile([128, nq * T], BF16, tag=f"at{nq}",
                              name=f"at_{bi}")
                for j in range(4):
                    nc.sync.dma_start(
                        out=st[32 * j:32 * j + D, :],
                        in_=stp_d[D * j:D * (j + 1), c0:c0 + nq * T],
                    )
                for j in range(4):
                    nc.sync.dma_start(
                        out=at[32 * j:32 * j + A, :],
                        in_=atp_d[A * j:A * (j + 1), c0:c0 + nq * T],
                    )
                blocks[bi] = (st, at)

            def _mm1(g):
                bi = QUAD_BLOCK[g]
                st, _ = blocks[bi]
                q = g - BLOCK_Q0[bi]
                ua, ub = (2 * g) % 3, (2 * g + 1) % 3
                for j in range(4):
                    dst = UN[ua] if j < 2 else UN[ub]
                    nc.tensor.matmul(
                        out=dst[:, T * (j % 2):T * (j % 2 + 1)],
                        lhsT=w1t[32 * j:32 * j + D, :],
                        rhs=st[32 * j:32 * j + D, T * q:T * (q + 1)],
                        start=True, stop=True,
                        tile_position=(32 * j, 0),
                    )

            def _act(g):
                ua, ub = (2 * g) % 3, (2 * g + 1) % 3
                h = hsb.tile([128, 4 * T], BF16, tag="h", name=f"h_{g}")
                if ub == ua + 1:
                    nc.scalar.activation(
                        out=h[:], in_=PS[:, 1024 * ua:1024 * ua + 2048],
                        func=mybir.ActivationFunctionType.Tanh,
                        bias=b1t[:], scale=1.0,
                    )
                else:
                    nc.scalar.activation(
                        out=h[:, 0:2 * T], in_=UN[ua],
                        func=mybir.ActivationFunctionType.Tanh,
                        bias=b1t[:], scale=1.0,
                    )
                    nc.scalar.activation(
                        out=h[:, 2 * T:4 * T], in_=UN[ub],
                        func=mybir.ActivationFunctionType.Tanh,
                        bias=b1t[:], scale=1.0,
                    )
                return h

            def _tail_quad(g, h):
                bi = QUAD_BLOCK[g]
                _, at = blocks[bi]
                q = g - BLOCK_Q0[bi]
                mu = MU[g % 2]
                for j in range(4):
                    nc.tensor.matmul(
                        out=mu[32 * j:32 * j + A, :],
                        lhsT=w2t[:],
                        rhs=h[:, T * j:T * (j + 1)],
                        start=True, stop=True,
                        tile_position=(0, 32 * j),
                        skip_group_check=True,
                    )
                # cross = sum_t v*c per partition (c = b2-a, bf16)
                sd = sdp.tile([128, T], BF16, tag="sd", name=f"sd_{g}")
                nc.vector.scalar_tensor_tensor(
                    out=sd[:], in0=mu[:], scalar=1.0,
                    in1=at[:, T * q:T * (q + 1)],
                    op0=mybir.AluOpType.mult, op1=mybir.AluOpType.mult,
                    accum_out=cr_sb[:, g:g + 1],
                )
                nc.vector.tensor_copy(ql_sb[:, g:g + 1], mu[:, T - 1:T])
                sts = bstp.tile([128, 6], F32, tag="bst", name=f"bst_{g}")
                nc.vector.bn_stats(out=sts[:], in_=mu[:])
                nc.vector.bn_aggr(out=mv_sb[:, 2 * g:2 * g + 2], in_=sts[:])

            for bi in range(4):
                _ensure_block(bi)
            # software pipeline, one quad ahead on mm1
            _mm1(0)
            hprev = None
            for g in range(NQ):
                if g + 1 < NQ:
                    bn = QUAD_BLOCK[g + 1]
                    if g + 1 == BLOCK_Q0[bn]:
                        _ensure_block(bn + 3)
                    _mm1(g + 1)
                h = _act(g)
                if hprev is not None:
                    _tail_quad(g - 1, hprev)
                hprev = h
                # stream outputs to keep the tail short
                if g % 16 == 8 and g > 16:
                    k = g // 16 - 1
                    nc.sync.dma_start(out=mv_d[:, 32 * k:32 * (k + 1)],
                                      in_=mv_sb[:, 32 * k:32 * (k + 1)])
                    nc.sync.dma_start(out=ql_d[:, 16 * k:16 * (k + 1)],
                                      in_=ql_sb[:, 16 * k:16 * (k + 1)])
                    nc.sync.dma_start(out=cr_d[:, 16 * k:16 * (k + 1)],
                                      in_=cr_sb[:, 16 * k:16 * (k + 1)])
            _tail_quad(NQ - 1, hprev)

            k = 3
            nc.sync.dma_start(out=mv_d[:, 32 * k:32 * (k + 1)],
                              in_=mv_sb[:, 32 * k:32 * (k + 1)])
            nc.sync.dma_start(out=ql_d[:, 16 * k:16 * (k + 1)],
                              in_=ql_sb[:, 16 * k:16 * (k + 1)])
            nc.sync.dma_start(out=cr_d[:, 16 * k:16 * (k + 1)],
                              in_=cr_sb[:, 16 * k:16 * (k + 1)])

    nc.finalize()
    return nc


_NC_CACHE = {}


def _get_program():
    if "nc" not in _NC_CACHE:
        _NC_CACHE["nc"] = _build_program()
    return _NC_CACHE["nc"]


def _make_consts(W1, b1, W2):
    w1full = np.zeros((128, HID), dtype=NP_BF16)
    for j in range(4):
        w1full[32 * j:32 * j + D, :] = W1.astype(NP_BF16)
    return {
        "w1full": w1full,
        "w2": np.ascontiguousarray(W2.astype(NP_BF16)),
        "b1col": np.ascontiguousarray(b1.astype(np.float32).reshape(HID, 1)),
    }


def kernel(states, actions, rewards, W1, b1, W2, b2, _run_kwargs=None):
    states = np.asarray(states, dtype=np.float32)
    actions = np.asarray(actions, dtype=np.float32)
    rewards = np.asarray(rewards, dtype=np.float32)
    W1 = np.asarray(W1, dtype=np.float32)
    b1 = np.asarray(b1, dtype=np.float32)
    W2 = np.asarray(W2, dtype=np.float32)
    b2 = np.asarray(b2, dtype=np.float32)

    consts = _make_consts(W1, b1, W2)

    # prepack per-core device layouts:
    #   st_pre[16j+dd, g*T+t] = states[core*SS + 4g+j, dd, t]   (bf16)
    #   at_pre[4j+d,  g*T+t] = b2[d] - actions[core*SS + 4g+j, d, t]  (bf16)
    st_all = states.reshape(N_CORES, SS // 4, 4, D, T)
    st_all = np.ascontiguousarray(st_all.transpose(0, 2, 3, 1, 4)).astype(NP_BF16)
    st_all = st_all.reshape(N_CORES, 64, NQ * T)
    aadj = b2[None, :, None] - actions
    at_all = aadj.reshape(N_CORES, SS // 4, 4, A, T)
    at_all = np.ascontiguousarray(at_all.transpose(0, 2, 3, 1, 4)).astype(NP_BF16)
    at_all = at_all.reshape(N_CORES, 16, NQ * T)

    in_maps = []
    for c in range(N_CORES):
        m = {"st_pre": st_all[c], "at_pre": at_all[c]}
        m.update(consts)
        in_maps.append(m)

    nc = _get_program()
    res = run_bass_kernel_spmd(nc, in_maps, core_ids=list(range(N_CORES)),
                               **(_run_kwargs or {}))
    results = res.results

    # host combine in float64
    C0 = -0.5 * A * np.log(2.0 * np.pi * SD_VAR)
    mx_pos = np.log(1.0 / (2.0 * MAX_POSITION))
    rew = rewards.astype(np.float64)
    R_all = rew.sum(axis=1)            # [S]
    rlast_all = rew[:, -1]             # [S]
    total = 0.0
    for c in range(N_CORES):
        mv = results[c]["mv"].astype(np.float64)      # [128, 2*NQ]
        qlv = results[c]["ql"].astype(np.float64)     # [128, NQ] = v at T-1
        crv = results[c]["cr"].astype(np.float64)     # [128, NQ] = sum v*c
        mean = mv[:, 0::2]
        var = mv[:, 1::2]
        sum_v2 = T * (var + mean * mean)              # [128, NQ]
        # partition p = 32j + d (d < A), sim s_local = 4g + j
        at64 = at_all[c].astype(np.float64).reshape(4, A, NQ, T)  # [j,d,g,t]
        c2 = (at64 ** 2).sum(axis=(1, 3))             # [j, g] = sum_{d,t} c^2
        clast = at64[:, :, :, -1]                     # [j, d, g]
        sel_v2 = sum_v2.reshape(4, 32, NQ)[:, :A, :]  # [j, d, g]
        sel_cr = crv.reshape(4, 32, NQ)[:, :A, :]
        sel_ql = qlv.reshape(4, 32, NQ)[:, :A, :]
        q_sum = (sel_v2 + 2.0 * sel_cr).sum(axis=1) + c2          # [j, g]
        q_sum = q_sum.T.reshape(SS)                   # s_local = 4g + j
        q_last = ((sel_ql + clast) ** 2).sum(axis=1).T.reshape(SS)
        sl = slice(SS * c, SS * (c + 1))
        L = -0.5 * q_sum / SD_VAR + T * C0
        ll_last = -0.5 * q_last / SD_VAR + C0
        A_sum = (R_all[sl] + rlast_all[sl]
                 - ALPHA * (L + ll_last) - T * mx_pos)
        total += np.sum(A_sum * L)
    out = np.float32(total / S)
    if _run_kwargs:
        _NC_CACHE["last_result"] = res
    return out


if __name__ == "__main__":
    rng = np.random.default_rng(0)
    inputs = {
        "states": rng.standard_normal((S, D, T), dtype=np.float32),
        "actions": rng.standard_normal((S, A, T), dtype=np.float32),
        "rewards": rng.standard_normal((S, T), dtype=np.float32),
        "W1": (rng.standard_normal((D, HID)) / np.sqrt(D)).astype(np.float32),
        "b1": np.zeros(HID, np.float32),
        "W2": (rng.standard_normal((HID, A)) / np.sqrt(HID)).astype(np.float32),
        "b2": np.zeros(A, np.float32),
    }
    print("result:", kernel(**inputs))
